# revision 19
# baseline (speedup 1.0000x reference)
"""Trainium2 Bass kernel for causal GQA attention (B=2, S=2048, D=2048,
H=32, KVH=8, hd=64) with RoPE and output projection, running SPMD on 8
NeuronCores.

Sharding: tensor-parallel over heads (4-way) x data-parallel over batch
(2-way).  Core c (b = c//4, k = c%4) handles batch b and heads
8k..8k+8 (kv heads 2k, 2k+1).  Attention outputs are AllGathered within
each batch group of 4 cores (split per q-tile so the collectives overlap
attention and the wo matmuls), after which each core computes a 512-wide
output-dim slice of the wo projection.  The host assembles the full
output, so no AllReduce is needed.

Layouts: everything lives in transposed [feature, seq] form so that the
head dim (the contraction dim of QK^T) sits on SBUF partitions and no
on-device transposes are required (except a cheap PE transpose for V).
All matmuls run in float32r (fp32 storage, reduced-precision multiply,
1 cyc/row).
"""

import numpy as np

DIM = 2048
S = 2048
B = 2
H = 32
KVH = 8
HD = 64
P = 128
HL = 8          # heads per core
QT = 512        # q tile (free dim of score matmuls)
NQT = S // QT   # 4
NKV = S // P    # 16 kv tiles of 128
DK = DIM // P   # 16 contraction tiles
ROPE_BASE = 10000.0
N_CORES = 8

_CACHE = {}


def _build():
    import concourse.bacc as bacc
    import concourse.tile as tile
    import concourse.mybir as mybir
    from concourse.masks import make_identity

    F32 = mybir.dt.float32
    F32R = mybir.dt.float32r
    Exp = mybir.ActivationFunctionType.Exp

    nc = bacc.Bacc("TRN2", target_bir_lowering=False, debug=False,
                   num_devices=N_CORES)

    xT = nc.dram_tensor("xT", [DIM, S], F32R, kind="ExternalInput").ap()
    wqT = nc.dram_tensor("wqT", [DIM, 512], F32R, kind="ExternalInput").ap()
    wkT = nc.dram_tensor("wkT", [DIM, 256], F32R, kind="ExternalInput").ap()
    wvT = nc.dram_tensor("wvT", [DIM, 128], F32R, kind="ExternalInput").ap()
    woT = nc.dram_tensor("woT", [DIM, 512], F32R, kind="ExternalInput").ap()
    cosT = nc.dram_tensor("cosT", [P, S], F32, kind="ExternalInput").ap()
    sinT = nc.dram_tensor("sinT", [P, S], F32, kind="ExternalInput").ap()
    maskT = nc.dram_tensor("maskT", [P, 4, QT], F32, kind="ExternalInput").ap()
    out_t = nc.dram_tensor("out_t", [512, S], F32, kind="ExternalOutput").ap()

    xT3 = xT.rearrange("(o p) s -> p o s", p=P)
    wqT3 = wqT.rearrange("(o p) f -> p o f", p=P)
    wkT3 = wkT.rearrange("(o p) f -> p o f", p=P)
    wvT3 = wvT.rearrange("(o p) f -> p o f", p=P)
    woT3 = woT.rearrange("(o p) f -> p o f", p=P)

    with tile.TileContext(nc) as tc:
        with (
            tc.tile_pool(name="pers", bufs=1) as pers,
            tc.tile_pool(name="ps", bufs=1, space="PSUM") as ps,
            tc.tile_pool(name="dram", bufs=1, space="DRAM") as dram,
        ):
            # ---- persistent tiles ----
            q_fin = [pers.tile([P, S], F32R, name=f"q_fin{m}") for m in range(4)]
            k_fin = [pers.tile([P, S], F32R, name=f"k_fin{g}") for g in range(2)]
            v1 = [pers.tile([P, NKV, P], F32R, name=f"v1_{g}") for g in range(2)]
            msk = pers.tile([P, 4, QT], F32, name="msk")

            cc_in = [dram.tile([512, QT], F32R, name=f"cc_in{t}")
                     for t in range(NQT)]
            cc_out = [dram.tile([4 * 512, QT], F32R, name=f"cc_out{t}")
                      for t in range(NQT)]

            # PSUM layout (8 banks): tag sc2 = 2 tiles of 2 banks,
            # tag pv = 2 tiles of 1 bank, tag wo2 = 1 tile of 2 banks.
            def sc2(name):
                return ps.tile([P, 2, QT], F32, tag="sc2", bufs=2, name=name)

            def pvb(name, shape=None, dtype=None):
                return ps.tile(shape or [P, QT], dtype or F32, tag="pv",
                               bufs=2, name=name)

            def wo2(name):
                return ps.tile([P, 2, QT], F32, tag="wo2", bufs=1, name=name)

            # ================= stage A/B: projections + RoPE =================
            with tc.tile_pool(name="pa", bufs=1) as pa:
                OCH = 2  # contraction 128-tiles per x DMA chunk
                xsl0 = pa.tile([P, OCH, QT], F32R, tag="xsl", bufs=6,
                               name="xsl0")
                nc.sync.dma_start(xsl0[:], xT3[:, 0:OCH, 0:QT])
                wq_sb = [pa.tile([P, DK, P], F32R, name=f"wq_sb{m}")
                         for m in range(4)]
                for m in range(4):
                    nc.sync.dma_start(wq_sb[m][:],
                                      wqT3[:, :, m * P:(m + 1) * P])
                wk_sb = pa.tile([P, DK, 256], F32R)
                wv_sb = pa.tile([P, DK, 128], F32R)
                nc.sync.dma_start(wk_sb[:], wkT3[:])
                nc.sync.dma_start(wv_sb[:], wvT3[:])
                cos_sb = pa.tile([P, S], F32)
                sin_sb = pa.tile([P, S], F32)
                nc.sync.dma_start(cos_sb[:], cosT[:])
                nc.sync.dma_start(sin_sb[:], sinT[:])
                nc.sync.dma_start(msk[:], maskT[:])
                ident_f = pa.tile([P, P], F32)
                ident = pa.tile([P, P], F32R)
                make_identity(nc, ident_f[:])
                nc.vector.tensor_copy(ident[:], ident_f[:])

                vT_raw = pa.tile([P, S], F32R)

                # ones columns of the PV stationary operand (memset on f32r
                # is not a valid ISA encoding, so memset f32 then copy)
                ones3 = pa.tile([P, NKV, HD], F32)
                nc.vector.memset(ones3[:], 1.0)
                for g in range(2):
                    nc.vector.tensor_copy(v1[g][:, :, 0:HD], ones3[:])

                for st in range(NQT):
                    ssl = slice(st * QT, (st + 1) * QT)
                    qa = sc2(f"qa{st}")
                    qb = sc2(f"qb{st}")
                    kk0 = pvb(f"kk0{st}")
                    kk1 = pvb(f"kk1{st}")
                    vv = wo2(f"vv{st}")
                    qps = [qa[:, 0, :], qa[:, 1, :], qb[:, 0, :], qb[:, 1, :]]
                    kps = [kk0[:], kk1[:]]
                    vps = vv[:, 0, :]
                    for oc in range(DK // OCH):
                        if st == 0 and oc == 0:
                            xsl = xsl0
                        else:
                            xsl = pa.tile([P, OCH, QT], F32R, tag="xsl",
                                          bufs=6, name="xsl")
                            nc.sync.dma_start(
                                xsl[:], xT3[:, oc * OCH:(oc + 1) * OCH, ssl])
                        for oo in range(OCH):
                            o = oc * OCH + oo
                            first = o == 0
                            last = o == DK - 1
                            for m in range(4):
                                nc.tensor.matmul(
                                    qps[m], wq_sb[m][:, o, :], xsl[:, oo, :],
                                    start=first, stop=last)
                            for g in range(2):
                                nc.tensor.matmul(
                                    kps[g],
                                    wk_sb[:, o, g * P:(g + 1) * P],
                                    xsl[:, oo, :],
                                    start=first, stop=last)
                            nc.tensor.matmul(
                                vps, wv_sb[:, o, :], xsl[:, oo, :],
                                start=first, stop=last)

                    # RoPE on q/k slices.  The psum->sbuf raw copies are
                    # split across DVE and ACT so the psum banks drain in
                    # parallel and the next s-tile's matmuls start sooner.
                    for i, (dst, src) in enumerate(
                            [(q_fin[m], qps[m]) for m in range(4)]
                            + [(k_fin[g], kps[g]) for g in range(2)]):
                        raw = pa.tile([P, QT], F32, tag="raw", bufs=4,
                                      name="raw")
                        if i % 2 == 0:
                            nc.vector.tensor_copy(raw[:], src)
                        else:
                            nc.scalar.copy(raw[:], src)
                        rot = pa.tile([P, QT], F32, tag="rot", bufs=3,
                                      name="rot")
                        for hh in range(2):
                            base = hh * HD
                            nc.sync.dma_start(rot[base:base + 32, :],
                                              raw[base + 32:base + 64, :])
                            nc.sync.dma_start(rot[base + 32:base + 64, :],
                                              raw[base:base + 32, :])
                        nc.vector.tensor_mul(rot[:], rot[:], sin_sb[:, ssl])
                        nc.vector.tensor_mul(raw[:], raw[:], cos_sb[:, ssl])
                        nc.vector.tensor_add(dst[:, ssl], raw[:], rot[:])
                    nc.scalar.copy(vT_raw[:, ssl], vps)

                # V1 assembly: transpose vT_raw 128x128 blocks
                for j in range(NKV):
                    pst = pvb(f"pst{j}", [P, P], F32R)
                    nc.tensor.transpose(pst[:], vT_raw[:, j * P:(j + 1) * P],
                                        ident[:])
                    for g in range(2):
                        nc.vector.tensor_copy(
                            v1[g][:, j, HD:P], pst[:, g * HD:(g + 1) * HD])

            # ========== stage C/D/E: attention + AllGather + wo ==========
            with tc.tile_pool(name="pc", bufs=1) as pc:
                wo_sb = pc.tile([P, DK, 512], F32R, name="wo_sb")
                nc.sync.dma_start(wo_sb[:], woT3[:])

                def wo_stage(t):
                    """wo projection for q tile t (after AllGather t).  The
                    cct load goes on the gpsimd queue: that queue already
                    blocks on collective completion, so the wait serializes
                    nothing else."""
                    qsl = slice(t * QT, (t + 1) * QT)
                    cc3 = cc_out[t][:].rearrange("(o p) s -> p o s", p=P)
                    cct = pc.tile([P, DK, QT], F32R, tag="cct", bufs=1,
                                  name="cct")
                    nc.gpsimd.dma_start(cct[:], cc3[:])
                    for dp in range(2):
                        pw = wo2(f"wo_{t}_{dp}")
                        for o in range(DK):
                            for dd in range(2):
                                d = dp * 2 + dd
                                nc.tensor.matmul(
                                    pw[:, dd, :],
                                    wo_sb[:, o, d * P:(d + 1) * P],
                                    cct[:, o, :],
                                    start=(o == 0), stop=(o == DK - 1))
                        for dd in range(2):
                            d = dp * 2 + dd
                            ot = pc.tile([P, QT], F32, tag="ot", bufs=2,
                                         name="ot")
                            nc.vector.tensor_copy(ot[:], pw[:, dd, :])
                            nc.sync.dma_start(out_t[d * P:(d + 1) * P, qsl],
                                              ot[:])

                t_order = list(range(NQT - 1, -1, -1))
                for ti, t in enumerate(t_order):
                    ngrp = 2 * (t + 1)
                    qsl = slice(t * QT, (t + 1) * QT)
                    for h in range(HL):
                        m, half, g = h // 2, h % 2, h // 4
                        pr = slice(half * HD, half * HD + HD)
                        pspv = pvb(f"pv_{t}_{h}")
                        e_tiles = []
                        for g2 in range(ngrp):
                            pss = sc2(f"ss_{t}_{h}_{g2}")
                            for i in range(2):
                                j = 2 * g2 + i
                                nc.tensor.matmul(
                                    pss[:, i, :],
                                    k_fin[g][pr, j * P:(j + 1) * P],
                                    q_fin[m][pr, qsl],
                                    start=True, stop=True)
                            e2 = pc.tile([P, 2, QT], F32R, tag="exp", bufs=5,
                                         name="e2")
                            nc.scalar.activation(e2[:], pss[:], Exp,
                                                 scale=0.125)
                            cpair = g2 - 2 * t
                            if cpair >= 0:
                                nc.vector.tensor_mul(
                                    e2[:], e2[:],
                                    msk[:, 2 * cpair:2 * cpair + 2, :])
                            e_tiles.append(e2)
                        for g2 in range(ngrp):
                            for i in range(2):
                                j = 2 * g2 + i
                                nc.tensor.matmul(
                                    pspv[:], v1[g][:, j, :],
                                    e_tiles[g2][:, i, :],
                                    start=(j == 0), stop=(j == 4 * t + 3))
                        # quick full copy so the pv psum bank releases while
                        # the normalize chain continues from SBUF
                        ocp = pc.tile([P, QT], F32, tag="ocp", bufs=3,
                                      name="ocp")
                        nc.vector.tensor_copy(ocp[:], pspv[:])
                        recip = pc.tile([1, QT], F32, tag="recip", bufs=2,
                                        name="recip")
                        nc.vector.reciprocal_approx_fast(recip[:],
                                                         ocp[0:1, :])
                        # broadcast 1/L to partitions 64:128 via a DRAM
                        # bounce (keeps gpsimd free for collective waits)
                        rb = dram.tile([1, QT], F32, tag="rb", bufs=2,
                                       name="rb")
                        nc.sync.dma_start(rb[:], recip[:])
                        bcast = pc.tile([P, QT], F32, tag="bcast", bufs=2,
                                        name="bcast")
                        nc.sync.dma_start(bcast[HD:P, :],
                                          rb[:].to_broadcast((HD, QT)))
                        o_sb = pc.tile([P, QT], F32R, tag="osb", bufs=2,
                                       name="o_sb")
                        nc.vector.tensor_mul(o_sb[HD:P, :], ocp[HD:P, :],
                                             bcast[HD:P, :])
                        nc.sync.dma_start(cc_in[t][h * HD:(h + 1) * HD, :],
                                          o_sb[HD:P, :])

                    # wo for the previous tile first (its AllGather has
                    # completed during this tile's attention), then trigger
                    # this tile's AllGather.
                    if ti >= 1:
                        wo_stage(t_order[ti - 1])
                    nc.gpsimd.collective_compute(
                        "AllGather",
                        mybir.AluOpType.bypass,
                        replica_groups=[[0, 1, 2, 3], [4, 5, 6, 7]],
                        ins=[cc_in[t][:].opt()],
                        outs=[cc_out[t][:].opt()],
                    )
                wo_stage(t_order[-1])

    nc.compile()
    return nc


def _prep_inputs(x, position_ids, wq, wk, wv, wo):
    x = np.asarray(x, dtype=np.float32)
    pos = np.asarray(position_ids).reshape(-1).astype(np.int64)
    wqTf = np.asarray(wq, dtype=np.float32).T
    wkTf = np.asarray(wk, dtype=np.float32).T
    wvTf = np.asarray(wv, dtype=np.float32).T
    woTf = np.asarray(wo, dtype=np.float32).T

    inv = 1.0 / (ROPE_BASE ** (np.arange(0, HD, 2, dtype=np.float32) / HD))
    freqs = np.outer(pos.astype(np.float32), inv)  # [S, 32]
    pidx = np.arange(P) % 32
    sign = np.where((np.arange(P) % HD) < 32, -1.0, 1.0).astype(np.float32)
    cosT = np.ascontiguousarray(np.cos(freqs)[:, pidx].T)          # [P, S]
    sinT = np.ascontiguousarray(np.sin(freqs)[:, pidx].T * sign[:, None])

    pg = np.arange(P)[:, None, None]
    cg = np.arange(4)[None, :, None]
    fg = np.arange(QT)[None, None, :]
    maskT = ((fg - pg - 128 * cg) >= 0).astype(np.float32)

    xT = [np.ascontiguousarray(x[b].T) for b in range(B)]

    in_maps = []
    for c in range(N_CORES):
        b, k = c // 4, c % 4
        wkT_loc = np.concatenate(
            [np.tile(wkTf[:, HD * (2 * k + g):HD * (2 * k + g + 1)], (1, 2))
             for g in range(2)], axis=1)
        in_maps.append({
            "xT": xT[b],
            "wqT": np.ascontiguousarray(wqTf[:, 512 * k:512 * (k + 1)]),
            "wkT": np.ascontiguousarray(wkT_loc),
            "wvT": np.ascontiguousarray(wvTf[:, 128 * k:128 * (k + 1)]),
            "woT": np.ascontiguousarray(woTf[:, 512 * k:512 * (k + 1)]),
            "cosT": cosT,
            "sinT": sinT,
            "maskT": maskT,
        })
    return in_maps


LAST_EXEC_NS = None


def kernel(x, position_ids, wq, wk, wv, wo, _trace=False):
    from concourse import bass_utils

    if "nc" not in _CACHE:
        _CACHE["nc"] = _build()
    nc = _CACHE["nc"]

    in_maps = _prep_inputs(x, position_ids, wq, wk, wv, wo)
    res = bass_utils.run_bass_kernel_spmd(
        nc, in_maps, core_ids=list(range(N_CORES)), trace=_trace)

    global LAST_EXEC_NS
    LAST_EXEC_NS = res.exec_time_ns

    out = np.empty((B, S, DIM), dtype=np.float32)
    for c in range(N_CORES):
        b, k = c // 4, c % 4
        out[b, :, 512 * k:512 * (k + 1)] = res.results[c]["out_t"].T
    return out


# revision 25
# speedup vs baseline: 1.1651x; 1.1651x over previous
"""Trainium2 Bass kernel for causal GQA attention (B=2, S=2048, D=2048,
H=32, KVH=8, hd=64) with RoPE and output projection, running SPMD on 8
NeuronCores.

Sharding: tensor-parallel over heads (4-way) x data-parallel over batch
(2-way).  Core c (b = c//4, k = c%4) handles batch b and heads
8k..8k+8 (kv heads 2k, 2k+1).  Attention outputs are AllGathered within
each batch group of 4 cores (split per q-tile so the collectives overlap
attention and the wo matmuls), after which each core computes a 512-wide
output-dim slice of the wo projection.  The host assembles the full
output, so no AllReduce is needed.

Layouts: everything lives in transposed [feature, seq] form so that the
head dim (the contraction dim of QK^T) sits on SBUF partitions and no
on-device transposes are required (except a cheap PE transpose for V).
All matmuls run in float32r (fp32 storage, reduced-precision multiply,
1 cyc/row).
"""

import numpy as np

DIM = 2048
S = 2048
B = 2
H = 32
KVH = 8
HD = 64
P = 128
HL = 8          # heads per core
QT = 512        # q tile (free dim of score matmuls)
NQT = S // QT   # 4
NKV = S // P    # 16 kv tiles of 128
DK = DIM // P   # 16 contraction tiles
ROPE_BASE = 10000.0
N_CORES = 8

_CACHE = {}


def _build():
    import concourse.bacc as bacc
    import concourse.tile as tile
    import concourse.mybir as mybir
    from concourse.masks import make_identity

    F32 = mybir.dt.float32
    F32R = mybir.dt.float32r
    Exp = mybir.ActivationFunctionType.Exp

    nc = bacc.Bacc("TRN2", target_bir_lowering=False, debug=False,
                   num_devices=N_CORES)

    xT = nc.dram_tensor("xT", [DIM, S], F32R, kind="ExternalInput").ap()
    wqT = nc.dram_tensor("wqT", [DIM, 512], F32R, kind="ExternalInput").ap()
    wkT = nc.dram_tensor("wkT", [DIM, 256], F32R, kind="ExternalInput").ap()
    wvT = nc.dram_tensor("wvT", [DIM, 128], F32R, kind="ExternalInput").ap()
    woT = nc.dram_tensor("woT", [DIM, 512], F32R, kind="ExternalInput").ap()
    cosT = nc.dram_tensor("cosT", [P, S], F32, kind="ExternalInput").ap()
    sinT = nc.dram_tensor("sinT", [P, S], F32, kind="ExternalInput").ap()
    maskT = nc.dram_tensor("maskT", [P, 4, QT], F32, kind="ExternalInput").ap()
    out_t = nc.dram_tensor("out_t", [512, S], F32, kind="ExternalOutput").ap()

    xT3 = xT.rearrange("(o p) s -> p o s", p=P)
    wqT3 = wqT.rearrange("(o p) f -> p o f", p=P)
    wkT3 = wkT.rearrange("(o p) f -> p o f", p=P)
    wvT3 = wvT.rearrange("(o p) f -> p o f", p=P)
    woT3 = woT.rearrange("(o p) f -> p o f", p=P)

    with tile.TileContext(nc) as tc:
        with (
            tc.tile_pool(name="pers", bufs=1) as pers,
            tc.tile_pool(name="ps", bufs=1, space="PSUM") as ps,
            tc.tile_pool(name="dram", bufs=1, space="DRAM") as dram,
        ):
            # ---- persistent tiles ----
            q_fin = [pers.tile([P, S], F32R, name=f"q_fin{m}") for m in range(4)]
            k_fin = [pers.tile([P, S], F32R, name=f"k_fin{g}") for g in range(2)]
            v1 = [pers.tile([P, NKV, P], F32R, name=f"v1_{g}") for g in range(2)]
            msk = pers.tile([P, 4, QT], F32, name="msk")

            cc_in = [[dram.tile([256, QT], F32R, name=f"cc_in{t}_{hh}")
                      for hh in range(2)] for t in range(NQT)]
            cc_out = [[dram.tile([4 * 256, QT], F32R, name=f"cc_out{t}_{hh}")
                       for hh in range(2)] for t in range(NQT)]

            # PSUM layout (8 banks): tag sc2 = 2 tiles of 2 banks,
            # tag pv = 2 tiles of 1 bank, tag wo2 = 1 tile of 2 banks.
            def sc2(name):
                return ps.tile([P, 2, QT], F32, tag="sc2", bufs=2, name=name)

            def pvb(name, shape=None, dtype=None):
                return ps.tile(shape or [P, QT], dtype or F32, tag="pv",
                               bufs=2, name=name)

            def wo2(name):
                return ps.tile([P, 2, QT], F32, tag="wo2", bufs=1, name=name)

            # ================= stage A/B: projections + RoPE =================
            with tc.tile_pool(name="pa", bufs=1) as pa:
                OCH = 2  # contraction 128-tiles per x DMA chunk
                xsl0 = pa.tile([P, OCH, QT], F32R, tag="xsl", bufs=6,
                               name="xsl0")
                nc.sync.dma_start(xsl0[:], xT3[:, 0:OCH, 0:QT])
                wq_sb = [pa.tile([P, DK, P], F32R, name=f"wq_sb{m}")
                         for m in range(4)]
                for m in range(4):
                    nc.sync.dma_start(wq_sb[m][:],
                                      wqT3[:, :, m * P:(m + 1) * P])
                wk_sb = pa.tile([P, DK, 256], F32R)
                wv_sb = pa.tile([P, DK, 128], F32R)
                nc.sync.dma_start(wk_sb[:], wkT3[:])
                nc.sync.dma_start(wv_sb[:], wvT3[:])
                cos_sb = pa.tile([P, S], F32)
                sin_sb = pa.tile([P, S], F32)
                nc.sync.dma_start(cos_sb[:], cosT[:])
                nc.sync.dma_start(sin_sb[:], sinT[:])
                nc.sync.dma_start(msk[:], maskT[:])
                ident_f = pa.tile([P, P], F32)
                ident = pa.tile([P, P], F32R)
                make_identity(nc, ident_f[:])
                nc.vector.tensor_copy(ident[:], ident_f[:])

                vT_raw = pa.tile([P, S], F32R)

                # ones columns of the PV stationary operand (memset on f32r
                # is not a valid ISA encoding, so memset f32 then copy)
                ones3 = pa.tile([P, NKV, HD], F32)
                nc.vector.memset(ones3[:], 1.0)
                for g in range(2):
                    nc.vector.tensor_copy(v1[g][:, :, 0:HD], ones3[:])

                for st in range(NQT):
                    ssl = slice(st * QT, (st + 1) * QT)
                    qa = sc2(f"qa{st}")
                    qb = sc2(f"qb{st}")
                    kk0 = pvb(f"kk0{st}")
                    kk1 = pvb(f"kk1{st}")
                    vv = wo2(f"vv{st}")
                    qps = [qa[:, 0, :], qa[:, 1, :], qb[:, 0, :], qb[:, 1, :]]
                    kps = [kk0[:], kk1[:]]
                    vps = vv[:, 0, :]
                    for oc in range(DK // OCH):
                        if st == 0 and oc == 0:
                            xsl = xsl0
                        else:
                            xsl = pa.tile([P, OCH, QT], F32R, tag="xsl",
                                          bufs=6, name="xsl")
                            nc.sync.dma_start(
                                xsl[:], xT3[:, oc * OCH:(oc + 1) * OCH, ssl])
                        for oo in range(OCH):
                            o = oc * OCH + oo
                            first = o == 0
                            last = o == DK - 1
                            for m in range(4):
                                nc.tensor.matmul(
                                    qps[m], wq_sb[m][:, o, :], xsl[:, oo, :],
                                    start=first, stop=last)
                            for g in range(2):
                                nc.tensor.matmul(
                                    kps[g],
                                    wk_sb[:, o, g * P:(g + 1) * P],
                                    xsl[:, oo, :],
                                    start=first, stop=last)
                            nc.tensor.matmul(
                                vps, wv_sb[:, o, :], xsl[:, oo, :],
                                start=first, stop=last)

                    # RoPE on q/k slices.  The psum->sbuf raw copies are
                    # split across DVE and ACT so the psum banks drain in
                    # parallel and the next s-tile's matmuls start sooner.
                    for i, (dst, src) in enumerate(
                            [(q_fin[m], qps[m]) for m in range(4)]
                            + [(k_fin[g], kps[g]) for g in range(2)]):
                        raw = pa.tile([P, QT], F32, tag="raw", bufs=4,
                                      name="raw")
                        if i % 2 == 0:
                            nc.vector.tensor_copy(raw[:], src)
                        else:
                            nc.scalar.copy(raw[:], src)
                        rot = pa.tile([P, QT], F32, tag="rot", bufs=3,
                                      name="rot")
                        for hh in range(2):
                            base = hh * HD
                            nc.sync.dma_start(rot[base:base + 32, :],
                                              raw[base + 32:base + 64, :])
                            nc.sync.dma_start(rot[base + 32:base + 64, :],
                                              raw[base:base + 32, :])
                        nc.vector.tensor_mul(rot[:], rot[:], sin_sb[:, ssl])
                        nc.vector.tensor_mul(raw[:], raw[:], cos_sb[:, ssl])
                        nc.vector.tensor_add(dst[:, ssl], raw[:], rot[:])
                    nc.scalar.copy(vT_raw[:, ssl], vps)

                # V1 assembly: transpose vT_raw 128x128 blocks
                for j in range(NKV):
                    pst = pvb(f"pst{j}", [P, P], F32R)
                    nc.tensor.transpose(pst[:], vT_raw[:, j * P:(j + 1) * P],
                                        ident[:])
                    for g in range(2):
                        nc.vector.tensor_copy(
                            v1[g][:, j, HD:P], pst[:, g * HD:(g + 1) * HD])

            # ========== stage C/D/E: attention + AllGather + wo ==========
            with tc.tile_pool(name="pc", bufs=1) as pc:
                wo_sb = pc.tile([P, DK, 512], F32R, name="wo_sb")
                nc.sync.dma_start(wo_sb[:], woT3[:])

                # Each q tile's attention output is gathered in two
                # half-tiles (heads 0-3 / 4-7) so the first AllGather fires
                # mid-tile.  wo for tile t runs per d-pair as the halves
                # arrive; the cct loads sit on the gpsimd queue, which is
                # the queue that blocks on collective completion anyway.
                # cct layout [p, hh, r, o2, s]: contraction tile
                # o = 4r + 2hh + o2 of the gathered [2048, QT] half pair.
                cct_tiles = {}

                def cct_load(t, hh):
                    if t not in cct_tiles:
                        cct_tiles[t] = pc.tile([P, 2, 4, 2, QT], F32R,
                                               tag="cct", bufs=1, name="cct")
                    cc3 = cc_out[t][hh][:].rearrange(
                        "(r o p) s -> p r o s", p=P, o=2)
                    nc.gpsimd.dma_start(cct_tiles[t][:, hh], cc3[:])

                def wo_half(t, hh, dp, pw):
                    """Accumulate gathered half hh into the wo psum for
                    output d-pair dp of q tile t."""
                    cct = cct_tiles[t]
                    for r in range(4):
                        for o2 in range(2):
                            o = 4 * r + 2 * hh + o2
                            for dd in range(2):
                                d = 2 * dp + dd
                                nc.tensor.matmul(
                                    pw[:, dd, :],
                                    wo_sb[:, o, d * P:(d + 1) * P],
                                    cct[:, hh, r, o2, :],
                                    start=(hh == 0 and r == 0 and o2 == 0),
                                    stop=(hh == 1 and r == 3 and o2 == 1))

                def wo_finish(t, dp, pw):
                    qsl = slice(t * QT, (t + 1) * QT)
                    for dd in range(2):
                        d = 2 * dp + dd
                        ot = pc.tile([P, QT], F32, tag="ot", bufs=2,
                                     name="ot")
                        nc.vector.tensor_copy(ot[:], pw[:, dd, :])
                        nc.sync.dma_start(out_t[d * P:(d + 1) * P, qsl],
                                          ot[:])

                def wo_rest(t, pw_a):
                    """Finish wo for tile t: d-pair 0 half 1, then all of
                    d-pair 1 (both halves are in SBUF by now)."""
                    wo_half(t, 1, 0, pw_a)
                    wo_finish(t, 0, pw_a)
                    pw_b = wo2(f"wo_{t}_b")
                    wo_half(t, 0, 1, pw_b)
                    wo_half(t, 1, 1, pw_b)
                    wo_finish(t, 1, pw_b)

                def attn_head(t, h):
                    ngrp = 2 * (t + 1)
                    qsl = slice(t * QT, (t + 1) * QT)
                    m, half, g = h // 2, h % 2, h // 4
                    pr = slice(half * HD, half * HD + HD)
                    pspv = pvb(f"pv_{t}_{h}")
                    e_tiles = []
                    for g2 in range(ngrp):
                        pss = sc2(f"ss_{t}_{h}_{g2}")
                        for i in range(2):
                            j = 2 * g2 + i
                            nc.tensor.matmul(
                                pss[:, i, :],
                                k_fin[g][pr, j * P:(j + 1) * P],
                                q_fin[m][pr, qsl],
                                start=True, stop=True)
                        e2 = pc.tile([P, 2, QT], F32R, tag="exp", bufs=5,
                                     name="e2")
                        nc.scalar.activation(e2[:], pss[:], Exp, scale=0.125)
                        cpair = g2 - 2 * t
                        if cpair >= 0:
                            nc.vector.tensor_mul(
                                e2[:], e2[:],
                                msk[:, 2 * cpair:2 * cpair + 2, :])
                        e_tiles.append(e2)
                    for g2 in range(ngrp):
                        for i in range(2):
                            j = 2 * g2 + i
                            nc.tensor.matmul(
                                pspv[:], v1[g][:, j, :],
                                e_tiles[g2][:, i, :],
                                start=(j == 0), stop=(j == 4 * t + 3))
                    # quick full copy so the pv psum bank releases while
                    # the normalize chain continues from SBUF
                    ocp = pc.tile([P, QT], F32, tag="ocp", bufs=3,
                                  name="ocp")
                    nc.vector.tensor_copy(ocp[:], pspv[:])
                    recip = pc.tile([1, QT], F32, tag="recip", bufs=2,
                                    name="recip")
                    nc.vector.reciprocal_approx_fast(recip[:], ocp[0:1, :])
                    # broadcast 1/L to partitions 64:128 via a DRAM bounce
                    # (keeps gpsimd free for collective waits)
                    rb = dram.tile([1, QT], F32, tag="rb", bufs=2, name="rb")
                    nc.sync.dma_start(rb[:], recip[:])
                    bcast = pc.tile([P, QT], F32, tag="bcast", bufs=2,
                                    name="bcast")
                    nc.sync.dma_start(bcast[HD:P, :],
                                      rb[:].to_broadcast((HD, QT)))
                    o_sb = pc.tile([P, QT], F32R, tag="osb", bufs=2,
                                   name="o_sb")
                    nc.vector.tensor_mul(o_sb[HD:P, :], ocp[HD:P, :],
                                         bcast[HD:P, :])
                    nc.sync.dma_start(
                        cc_in[t][h // 4][(h % 4) * HD:(h % 4 + 1) * HD, :],
                        o_sb[HD:P, :])

                def trig_ag(t, hh):
                    nc.gpsimd.collective_compute(
                        "AllGather",
                        mybir.AluOpType.bypass,
                        replica_groups=[[0, 1, 2, 3], [4, 5, 6, 7]],
                        ins=[cc_in[t][hh][:].opt()],
                        outs=[cc_out[t][hh][:].opt()],
                    )

                pw_prev = None
                for t in range(NQT):
                    if t >= 1:
                        cct_load(t - 1, 1)   # gpsimd: waits AG(t-1,1)
                    for h in range(4):
                        attn_head(t, h)
                    trig_ag(t, 0)
                    if t >= 1:
                        wo_rest(t - 1, pw_prev)
                    for h in range(4, HL):
                        attn_head(t, h)
                    cct_load(t, 0)           # gpsimd: waits AG(t,0)
                    trig_ag(t, 1)
                    pw_prev = wo2(f"wo_{t}_a")
                    wo_half(t, 0, 0, pw_prev)   # runs during AG(t,1)
                cct_load(NQT - 1, 1)
                wo_rest(NQT - 1, pw_prev)

    nc.compile()
    return nc


def _prep_inputs(x, position_ids, wq, wk, wv, wo):
    x = np.asarray(x, dtype=np.float32)
    pos = np.asarray(position_ids).reshape(-1).astype(np.int64)
    wqTf = np.asarray(wq, dtype=np.float32).T
    wkTf = np.asarray(wk, dtype=np.float32).T
    wvTf = np.asarray(wv, dtype=np.float32).T
    woTf = np.asarray(wo, dtype=np.float32).T

    inv = 1.0 / (ROPE_BASE ** (np.arange(0, HD, 2, dtype=np.float32) / HD))
    freqs = np.outer(pos.astype(np.float32), inv)  # [S, 32]
    pidx = np.arange(P) % 32
    sign = np.where((np.arange(P) % HD) < 32, -1.0, 1.0).astype(np.float32)
    cosT = np.ascontiguousarray(np.cos(freqs)[:, pidx].T)          # [P, S]
    sinT = np.ascontiguousarray(np.sin(freqs)[:, pidx].T * sign[:, None])

    pg = np.arange(P)[:, None, None]
    cg = np.arange(4)[None, :, None]
    fg = np.arange(QT)[None, None, :]
    maskT = ((fg - pg - 128 * cg) >= 0).astype(np.float32)

    xT = [np.ascontiguousarray(x[b].T) for b in range(B)]

    in_maps = []
    for c in range(N_CORES):
        b, k = c // 4, c % 4
        wkT_loc = np.concatenate(
            [np.tile(wkTf[:, HD * (2 * k + g):HD * (2 * k + g + 1)], (1, 2))
             for g in range(2)], axis=1)
        in_maps.append({
            "xT": xT[b],
            "wqT": np.ascontiguousarray(wqTf[:, 512 * k:512 * (k + 1)]),
            "wkT": np.ascontiguousarray(wkT_loc),
            "wvT": np.ascontiguousarray(wvTf[:, 128 * k:128 * (k + 1)]),
            "woT": np.ascontiguousarray(woTf[:, 512 * k:512 * (k + 1)]),
            "cosT": cosT,
            "sinT": sinT,
            "maskT": maskT,
        })
    return in_maps


LAST_EXEC_NS = None


def kernel(x, position_ids, wq, wk, wv, wo, _trace=False):
    from concourse import bass_utils

    if "nc" not in _CACHE:
        _CACHE["nc"] = _build()
    nc = _CACHE["nc"]

    in_maps = _prep_inputs(x, position_ids, wq, wk, wv, wo)
    res = bass_utils.run_bass_kernel_spmd(
        nc, in_maps, core_ids=list(range(N_CORES)), trace=_trace)

    global LAST_EXEC_NS
    LAST_EXEC_NS = res.exec_time_ns

    out = np.empty((B, S, DIM), dtype=np.float32)
    for c in range(N_CORES):
        b, k = c // 4, c % 4
        out[b, :, 512 * k:512 * (k + 1)] = res.results[c]["out_t"].T
    return out
